# revision 1
# baseline (speedup 1.0000x reference)
"""AttentionHead kernel for 8 Trainium2 NeuronCores.

Problem: B=4, S=2048, DIN=1024, DOUT=128 single-head attention with a
key-padding mask and a sqrt(S) score scale (see the module reference).

Sharding: 8 cores = 4 batches x 2 query-halves. Each core computes the
full K/V for its batch (redundantly with its pair core) and the
attention output for its 1024 query rows. Inputs are staged host-side
per core as x^T in fp16 (with the core's query-half columns permuted to
the front so a single SPMD program serves all cores), transposed fp16
weights, and a per-key mask bias.

All matmuls run with fp16 operands (exact FP22 multiplies, fp32 PSUM
accumulation; measured end-to-end rel err ~5e-4). fp32r would halve the
input-rounding error but produces all-zero output on this toolchain.

Per-core dataflow:
  1. Warmup matmuls (HAM un-throttle) while the first DMAs land.
  2. x^T [1024d, 2048s] DMA'd in 8 chunks; K^T/Q^T projections run
     d-chunk-outer so the PE chases the DMA; PSUM->SBUF copies (+bias,
     fp16 cast) split across DVE and ACT halves.
  3. scores^T[k,q] = K^T_tile.T @ Q^T per k-tile (full 1024-query
     width, two 512-col matmuls into one 2-bank PSUM tile), then one
     [128,1024] exp on ACT with the mask folded into the per-partition
     bias and 1/sqrt(S) folded into the activation scale. ACT's 16-exp
     stream is the attention-phase critical path.
  4. V in natural [s,o] layout via x^T-stationary matmuls (N=128),
     with a ones column appended so the softmax denominator rides the
     context matmul; V bias added via a partition-broadcast tile.
  5. context[q, o|den] accumulates 16 k-tile matmuls per 128-query
     tile (P^T stationary); normalize = DVE reciprocal + per-partition
     multiply; output DMA'd in pieces as query tiles complete.
"""

import sys

for _p in ("/opt/trn_rl_repo", "/root/.axon_site",
           "/root/.axon_site/_ro/trn_rl_repo", "/root/.axon_site/_ro/pypackages"):
    if _p not in sys.path:
        sys.path.insert(0, _p)

import numpy as np

B, S, DIN, DOUT = 4, 2048, 1024, 128
NCORES = 8
SH = S // 2          # seq half (query rows per core)
DC = DIN // 128      # d chunks (8)
KT_TILES = S // 128  # k tiles (16)
QB = SH // 512       # query blocks per core (2)
SCALE = 1.0 / float(np.sqrt(np.float32(S)))
MASK_BIAS = -60.0    # exp(-60) ~ 8.8e-27: numerically zero vs unmasked sum

_PROGRAM = None


def _build_program(reps=1):
    import concourse.bass as bass
    import concourse.mybir as mybir
    import concourse.tile as tile
    from concourse import bacc
    from concourse.masks import make_identity
    from contextlib import ExitStack

    f32 = mybir.dt.float32
    f32r = mybir.dt.float16  # PE input dtype (fp32r is broken on this toolchain)

    nc = bacc.Bacc(None, target_bir_lowering=False)

    xt_d = nc.dram_tensor("xt", [DIN, S], f32r, kind="ExternalInput")
    wk2_d = nc.dram_tensor("wk2", [DIN, DOUT], f32r, kind="ExternalInput")
    wq2_d = nc.dram_tensor("wq2", [DIN, DOUT], f32r, kind="ExternalInput")
    wv_d = nc.dram_tensor("wv2", [DIN, DOUT], f32r, kind="ExternalInput")
    bq_d = nc.dram_tensor("bq", [DOUT, 1], f32, kind="ExternalInput")
    bk_d = nc.dram_tensor("bk", [DOUT, 1], f32, kind="ExternalInput")
    bv_d = nc.dram_tensor("bv", [DOUT, 1], f32, kind="ExternalInput")
    mb_d = nc.dram_tensor("mbias", [128, KT_TILES], f32, kind="ExternalInput")
    ones_d = nc.dram_tensor("ones", [128, 4], f32r, kind="ExternalInput")
    out_d = nc.dram_tensor("out", [SH, DOUT], f32, kind="ExternalOutput")

    with ExitStack() as ctx:
        tc = ctx.enter_context(tile.TileContext(nc))
        consts = ctx.enter_context(tc.tile_pool(name="consts", bufs=1))
        xtp = ctx.enter_context(tc.tile_pool(name="xtp", bufs=DC))
        kqv = ctx.enter_context(tc.tile_pool(name="kqv", bufs=1))
        vnp = ctx.enter_context(tc.tile_pool(name="vnp", bufs=KT_TILES))
        ptp = ctx.enter_context(tc.tile_pool(name="ptp", bufs=KT_TILES))
        outp = ctx.enter_context(tc.tile_pool(name="outp", bufs=1))
        misc = ctx.enter_context(tc.tile_pool(name="misc", bufs=8))

        # ---- constants: wq/wk first (gate the first matmuls), then the
        # rest on the ACT HWDGE queue so they don't delay the x^T chunks.
        wk_sb = consts.tile([128, DC, DOUT], f32r, tag="wk", name="wk")
        nc.sync.dma_start(wk_sb, wk2_d.rearrange("(c p) o -> p c o", p=128))
        wq_sb = consts.tile([128, DC, DOUT], f32r, tag="wq", name="wq")
        nc.scalar.dma_start(wq_sb, wq2_d.rearrange("(c p) o -> p c o", p=128))
        wv_sb = consts.tile([128, DC, DOUT], f32r, tag="wv", name="wv")
        nc.scalar.dma_start(
            wv_sb, wv_d.rearrange("(c p) o -> p c o", p=128))
        w_sb = {"wq": wq_sb, "wk": wk_sb, "wv": wv_sb}
        mb_sb, ones_sb, bv_bc = {}, {}, {}
        b_sb = {}

        psA = ctx.enter_context(tc.tile_pool(name="psA", bufs=3, space="PSUM"))
        psM = ctx.enter_context(tc.tile_pool(name="psM", bufs=2, space="PSUM"))

        def body():
            # HAM warmup: dummy matmuls with no DMA dependency keep the PE
            # busy (and un-throttled) while the first x^T chunks land.
            dummy = misc.tile([128, 256], f32r, tag="dummy", name="dummy")
            nc.vector.memset(dummy, 0.5)
            for i in range(12):
                psw = psM.tile([128, 132], f32, tag="psM", name=f"warm{i}")
                nc.tensor.matmul(psw[:, 0:128], dummy[:, 0:128],
                                 dummy[:, 0:128], start=True, stop=True)

            # ---- x^T load (8 chunks of [128, 2048]) --------------------
            xt_sb = []
            for c in range(DC):
                t = xtp.tile([128, S], f32r, tag="xt", name=f"xt{c}")
                nc.sync.dma_start(t, xt_d[c * 128:(c + 1) * 128, :])
                xt_sb.append(t)
            if not b_sb:
                for name, d in (("bq", bq_d), ("bk", bk_d), ("bv", bv_d)):
                    t = consts.tile([DOUT, 1], f32, tag=name, name=name)
                    nc.scalar.dma_start(t, d[:, :])
                    b_sb[name] = t
                mb_sb["t"] = consts.tile([128, KT_TILES], f32, tag="mbias",
                                         name="mbias")
                nc.scalar.dma_start(mb_sb["t"], mb_d[:, :])
                ones_sb["t"] = consts.tile([128, 4], f32r, tag="ones",
                                           name="ones")
                nc.scalar.dma_start(ones_sb["t"], ones_d[:, :])
                bv_bc["t"] = consts.tile([128, DOUT], f32, tag="bv_bc",
                                         name="bv_bc")
                nc.gpsimd.dma_start(
                    out=bv_bc["t"],
                    in_=bass.AP(tensor=bv_d, offset=0,
                                ap=[[0, 128], [1, DOUT]]))

            KTh = [kqv.tile([128, 1024], f32r, tag=f"KT{i}", name=f"KT{i}")
                   for i in range(2)]
            QTh = [kqv.tile([128, 512], f32r, tag=f"QT{i}", name=f"QT{i}")
                   for i in range(2)]

            # K^T and Q^T, d-chunk outer so compute overlaps the x^T DMA.
            psK = [psA.tile([128, 1024], f32, tag="psA", name=f"psK{i}")
                   for i in range(2)]
            psQ = psA.tile([128, 1024], f32, tag="psA", name="psQ")
            for c in range(DC):
                for sb in range(4):
                    nc.tensor.matmul(
                        psK[sb // 2][:, (sb % 2) * 512:(sb % 2 + 1) * 512],
                        w_sb["wk"][:, c, :],
                        xt_sb[c][:, sb * 512:(sb + 1) * 512],
                        start=(c == 0), stop=(c == DC - 1))
                for sb in range(2):
                    nc.tensor.matmul(
                        psQ[:, sb * 512:(sb + 1) * 512],
                        w_sb["wq"][:, c, :],
                        xt_sb[c][:, sb * 512:(sb + 1) * 512],
                        start=(c == 0), stop=(c == DC - 1))
            def _copy_split(dst, src_ps, bias):
                nc.vector.tensor_scalar_add(
                    dst[:, 0:512], src_ps[:, 0:512], bias)
                nc.scalar.activation(
                    dst[:, 512:1024], src_ps[:, 512:1024],
                    mybir.ActivationFunctionType.Identity, bias=bias)

            nc.vector.tensor_scalar_add(QTh[0], psQ[:, 0:512], b_sb["bq"])
            nc.scalar.activation(
                QTh[1], psQ[:, 512:1024],
                mybir.ActivationFunctionType.Identity, bias=b_sb["bq"])
            _copy_split(KTh[0], psK[0], b_sb["bk"])
            _copy_split(KTh[1], psK[1], b_sb["bk"])

            # scores^T + exp over the full 1024-query range, per k-tile.
            # Emitted before the V projection so ACT (exp) fills while the
            # PE moves on to V.
            PT = []
            for kt in range(KT_TILES):
                pss = psA.tile([128, 1024], f32, tag="psA", name=f"psS{kt}")
                for qh in range(2):
                    nc.tensor.matmul(
                        pss[:, qh * 512:(qh + 1) * 512],
                        KTh[kt // 8][:, (kt % 8) * 128:(kt % 8 + 1) * 128],
                        QTh[qh],
                        start=True, stop=True)
                pt = ptp.tile([128, 1024], f32r, tag="pt", name=f"pt{kt}")
                nc.scalar.activation(
                    pt, pss, mybir.ActivationFunctionType.Exp,
                    bias=mb_sb["t"][:, kt:kt + 1], scale=SCALE)
                PT.append(pt)

            # V in natural [s, o] layout directly (x^T tiles stationary),
            # with the ones column appended for the denominator.
            VN = []
            for kt in range(KT_TILES):
                psv = psM.tile([128, 132], f32, tag="psM", name=f"psV{kt}")
                for c in range(DC):
                    nc.tensor.matmul(
                        psv[:, 0:128],
                        xt_sb[c][:, kt * 128:(kt + 1) * 128],
                        w_sb["wv"][:, c, :],
                        start=(c == 0), stop=(c == DC - 1))
                vt = vnp.tile([128, 132], f32r, tag="vn", name=f"vn{kt}")
                nc.vector.tensor_tensor(
                    vt[:, 0:128], psv[:, 0:128], bv_bc["t"], mybir.AluOpType.add)
                nc.vector.tensor_copy(out=vt[:, 128:132], in_=ones_sb["t"])
                VN.append(vt)

            # ---- context + normalize -----------------------------------
            # psc slots alternate between the (now free) scores pool and
            # psM so the DVE normalize never stalls the PE accumulations.
            out_r = out_d.rearrange("(t p) o -> p t o", p=128)
            OUT = outp.tile([128, SH // 128, DOUT], f32, tag="out")
            n_q2 = SH // 128
            for q2 in range(n_q2):
                if q2 in (0, 1, 5, 6):
                    psc = psM.tile([128, 132], f32, tag="psM", name=f"psC{q2}")
                else:
                    psc = psA.tile([128, 1024], f32, tag="psA",
                                   name=f"psC{q2}")[:, 0:132]
                for kt in range(KT_TILES):
                    nc.tensor.matmul(
                        psc,
                        PT[kt][:, q2 * 128:(q2 + 1) * 128],
                        VN[kt][:, 0:132],
                        start=(kt == 0), stop=(kt == KT_TILES - 1))
                drec = misc.tile([128, 1], f32, tag="drec", name=f"drec{q2}")
                nc.vector.reciprocal(drec, psc[:, 128:129])
                nc.vector.tensor_scalar_mul(
                    OUT[:, q2, :], psc[:, 0:128], drec)
                if q2 % 2 == 1:
                    nc.scalar.dma_start(
                        out_r[:, q2 - 1:q2 + 1, :], OUT[:, q2 - 1:q2 + 1, :])

        if reps == 1:
            body()
        else:
            with tc.For_i(0, reps, 1):
                body()

    nc.finalize()
    return nc


def _get_program():
    global _PROGRAM
    if _PROGRAM is None:
        _PROGRAM = _build_program()
    return _PROGRAM


def _stage_inputs(inputs):
    x = np.asarray(inputs["input_tensor"], dtype=np.float32)
    mask = np.asarray(inputs["attention_mask"]).astype(bool)
    ws = {k: np.asarray(inputs[k], dtype=np.float32)
          for k in ("wq", "wk", "wv")}
    bs = {k: np.asarray(inputs[k], dtype=np.float32).reshape(DOUT, 1)
          for k in ("bq", "bk", "bv")}
    wq2 = np.ascontiguousarray(ws["wq"].T).astype(np.float16)
    wk2 = np.ascontiguousarray(ws["wk"].T).astype(np.float16)
    wv2 = np.ascontiguousarray(ws["wv"].T).astype(np.float16)

    in_maps = []
    for c in range(NCORES):
        b, h = divmod(c, 2)
        xt = x[b].T  # [DIN, S]
        # Permute the core's query-half columns to the front so the same
        # program computes Q^T from columns [0, SH). K/V/mask use the same
        # permuted key order; the context sum is order-invariant.
        xt = np.ascontiguousarray(
            np.concatenate([xt[:, h * SH:(h + 1) * SH],
                            xt[:, (1 - h) * SH:(2 - h) * SH]],
                           axis=1)).astype(np.float16)
        m = mask[b, 0]
        mp = np.concatenate([m[h * SH:(h + 1) * SH],
                             m[(1 - h) * SH:(2 - h) * SH]])
        mbias = np.where(mp, np.float32(MASK_BIAS), np.float32(0.0))
        mbias = np.ascontiguousarray(
            mbias.reshape(KT_TILES, 128).T).astype(np.float32)
        in_maps.append({
            "xt": xt,
            "wq2": wq2, "wk2": wk2, "wv2": wv2,
            "bq": bs["bq"], "bk": bs["bk"], "bv": bs["bv"],
            "ones": np.ones((128, 4), dtype=np.float16),
            "mbias": mbias,
        })
    return in_maps


def run(inputs, **spmd_kwargs):
    """Run on 8 cores; returns (full_output, BassKernelResults)."""
    from concourse import bass_utils

    nc = _get_program()
    in_maps = _stage_inputs(inputs)
    res = bass_utils.run_bass_kernel_spmd(
        nc, in_maps, core_ids=list(range(NCORES)), **spmd_kwargs)
    out = np.empty((B, S, DOUT), dtype=np.float32)
    for c in range(NCORES):
        b, h = divmod(c, 2)
        out[b, h * SH:(h + 1) * SH, :] = res.results[c]["out"]
    return out, res


def kernel(**inputs) -> np.ndarray:
    return run(inputs)[0]

